# revision 25
# baseline (speedup 1.0000x reference)
"""Distributed Trainium2 Bass kernel for AlignmentContrastiveLoss (v2).

Reference computation (B=256, L_im=37, L_s=33, D=1024):
    im  = l2norm(im_set)[:, 1:, :]   masked by im_len-1     [B, 36, D]
    s   = l2norm(s_seq)[:, 1:-2, :]  masked by s_len-3      [B, 30, D]
    align[b,c,i,j] = im[b,i] . s[c,j]   (masked entries -> 0)
    scores[b,c] = sum_j max_i align[b,c,i,j]
    loss = sum_b relu(M + max_{c!=b} scores[b,c] - scores[b,b])
         + sum_c relu(M + max_{b!=c} scores[b,c] - scores[c,c])

v2 strategy (vs the bf16 full-density baseline):
  * s side is compacted on the host: only the valid (c, j) word rows are
    shipped (plus zero padding to 128-row tiles, each 128-sentence half
    padded separately so every tile maps to one half).  NT drops 60 -> ~36.
  * im side: each image's valid rows are padded up to R in {12,...,36}
    (multiple of G=6, with >=1 zero row unless im_l==36 so the reference's
    max-includes-zero clamp is preserved).  Images are rank-sorted by R and
    dealt round-robin to the 8 cores, so all cores share one R "template"
    (SPMD requires identical reduce shapes); rows ~9216 -> ~5300+pad/core... 888.
  * The big einsum runs in fp8 e4m3 with DoubleRow perf mode (K=256 per
    instruction at 0.5 cycles/row): ~4x fewer PE cycles than bf16.
    s is quantized host-side (raw values, |s| << 240); im is normalized on
    device, scaled x16 and cast to fp8; the 1/16 folds into the s scale.
  * No collectives at all: s norms come from the diagonal of per-tile
    fp8 Gram matmuls on the PE (each core loads all s tiles anyway), and
    the final cross-core combine (max/sum over 8 cores' 128x6 payloads)
    happens on the host - that is the gather/unshard step.
  * The max-over-i reduces are split between DVE (direct PSUM reads) and
    GpSimd (via a ScalarE PSUM->SBUF bf16 copy), since DVE alone would be
    the bottleneck at ~1.12 ns/elem.

The device program shape depends only on (im_len, s_len); build_nc is
cached on those layout parameters and recompiled if they change.
"""

import os
import sys

import numpy as np
import ml_dtypes

for _p in ("/opt/trn_rl_repo", "/root/.axon_site/_ro/trn_rl_repo"):
    if os.path.isdir(_p) and _p not in sys.path:
        sys.path.append(_p)

import concourse.bass as bass
import concourse.mybir as mybir
import concourse.tile as tile
from concourse import bacc
from concourse.bass_utils import run_bass_kernel_spmd


def _ensure_axon_hooks():
    """Some agent images ship an ``antenv`` without ``axon_hooks``, but
    bass_utils hard-imports it when trace=True.  Provide the registry and,
    when libaxon_pjrt.so is available, the real NTFF profile hook."""
    import types

    try:
        import antenv.axon_hooks  # noqa: F401
        return
    except ImportError:
        pass
    try:
        import antenv
    except ImportError:
        return
    mod = types.ModuleType("antenv.axon_hooks")
    mod._hook = None
    mod.set_axon_ntff_profile_hook = lambda h: setattr(mod, "_hook", h)
    mod.get_axon_ntff_profile_hook = lambda: mod._hook
    sys.modules["antenv.axon_hooks"] = mod
    antenv.axon_hooks = mod
    so_path = "/opt/axon/libaxon_pjrt.so"
    try:
        import trn_agent_boot.trn_boot as _tb
        if os.path.exists(so_path):
            mod._hook = _tb._ntff_profile_via_ctypes(so_path)
    except Exception:
        pass


_ensure_axon_hooks()

F32 = mybir.dt.float32
F32R = mybir.dt.float32r
BF16 = mybir.dt.bfloat16
F8 = mybir.dt.float8e4
I32 = mybir.dt.int32
AX = mybir.AxisListType
ALU = mybir.AluOpType
ACT = mybir.ActivationFunctionType
DR = mybir.MatmulPerfMode.DoubleRow

NCORES = 8
B, LI, LS, D = 256, 36, 30, 1024
KC = D // 128               # 8 contraction chunks of 128
G = 6                       # im row-padding granularity
MARGIN, EPS, NEG = 0.2, 1e-12, -1.0e9

LAST_RESULT = None  # BassKernelResults of the most recent run (for test harness)

# Dedup redundant PE weight loads: bass lowering splits every matmul into a
# standalone Ldweights + non-self-loading Matmult, but emits one Ldweights
# per matmul even when consecutive matmuls share the same stationary
# operand.  We post-process the BIR json and drop a generated Ldweights
# (no semaphore waits/updates) when the weights signature matches what the
# PE already has loaded.  This is the dominant PE cost for DoubleRow fp8
# (256-column loads at ~112 ns apiece).
LDW_DEDUP = os.environ.get("LDW_DEDUP", "1") == "1"


def _dedup_ldweights_json(js_bytes):
    import json as _json

    j = _json.loads(js_bytes)
    dropped = 0
    for fn in j.get("functions", []):
        for blk in fn.get("blocks", []):
            insts = blk.get("instructions")
            if not insts:
                continue
            out = []
            loaded = None
            for x in insts:
                if x.get("engine") != "PE":
                    out.append(x)
                    continue
                op = x.get("opcode")
                if op == "Ldweights":
                    sig = _json.dumps(
                        [x.get("ins"), x.get("perf_mode"),
                         x.get("tile_size"), x.get("tile_position"),
                         x.get("is_transpose")], sort_keys=True)
                    sync = x.get("sync_info") or {}
                    if (sig == loaded and not sync.get("on_wait")
                            and not sync.get("on_update")):
                        dropped += 1
                        continue
                    loaded = sig
                    out.append(x)
                elif op == "Matmult":
                    if x.get("ldweights") is not False:
                        loaded = None  # self-loading matmul clobbers weights
                    out.append(x)
                else:
                    loaded = None
                    out.append(x)
            blk["instructions"] = out
    return _json.dumps(j).encode(), dropped


# ---------------------------------------------------------------------------
# layout planning (data-dependent, host side)
# ---------------------------------------------------------------------------

class Plan:
    pass


def plan_layout(im_l, s_l):
    p = Plan()
    # ---- s side: per-half compacted row lists ----
    p.NT_h = []
    p.srows = []            # per half: list of (c, j) or None (pad)
    for h in (0, 1):
        rows = [(c, j) for c in range(128 * h, 128 * h + 128)
                for j in range(int(s_l[c]))]
        nt = -(-len(rows) // 128)
        rows = rows + [None] * (nt * 128 - len(rows))
        p.NT_h.append(nt)
        p.srows.append(rows)
    p.NT = p.NT_h[0] + p.NT_h[1]

    # ---- im side: R template shared across cores ----
    R = np.where(im_l >= LI, LI,
                 (G * np.ceil((im_l + 1) / G)).astype(np.int64)).astype(np.int64)
    order = np.argsort(-R, kind="stable")
    p.order = order                       # slot i of core m -> image order[8i+m]
    p.template = [int(R[order[8 * i]]) for i in range(32)]
    # runs of equal R in the (descending) template
    runs = []
    i = 0
    while i < 32:
        j = i
        while j < 32 and p.template[j] == p.template[i]:
            j += 1
        runs.append({"mxoff": i, "n": j - i, "R": p.template[i]})
        i = j
    # pack runs into PSUM banks; the LAST bin reserves 128 cols for the
    # gram matmul (cols 384:512) so the whole per-tile accumulation
    # tensor is nbins banks.  Splitting a run costs one extra segment.
    for nbins in range(2, 9):
        caps = [512] * (nbins - 1) + [384]
        segs = _pack_runs(runs, caps)
        if segs is not None:
            break
    p.nbins = nbins
    p.segs = segs
    for s in segs:
        s["eng"] = "dve"
    p.NR = sum(s["n"] * s["R"] for s in segs)
    p.NRT = -(-p.NR // 128)
    # im slot row offsets: global row index = bin*512... no - rows are
    # packed per-bin contiguously in imt column space: bin bi occupies
    # imt cols [binoff[bi], binoff[bi]+used[bi])
    used = [0] * nbins
    for s in segs:
        used[s["bin"]] = max(used[s["bin"]], s["off"] + s["n"] * s["R"])
    binoff = [0] * nbins
    for bi in range(1, nbins):
        binoff[bi] = binoff[bi - 1] + used[bi - 1]
    p.bin_used, p.bin_off = used, binoff
    p.slot_off = [0] * 32
    for s in segs:
        for t in range(s["n"]):
            p.slot_off[s["mxoff"] + t] = (binoff[s["bin"]] + s["off"]
                                          + t * s["R"])
    return p


def _pack_runs(runs, caps):
    """First-fit-decreasing of template runs into bins with capacities
    ``caps``; runs may split at image boundaries.  Returns segment list
    or None if it does not fit."""
    free = list(caps)
    segs = []
    for r in sorted(runs, key=lambda r: -r["n"] * r["R"]):
        n, R, mxoff = r["n"], r["R"], r["mxoff"]
        # whole-run first fit
        placed = False
        for bi in range(len(caps)):
            if n * R <= free[bi]:
                segs.append({"bin": bi, "off": caps[bi] - free[bi],
                             "n": n, "R": R, "mxoff": mxoff})
                free[bi] -= n * R
                placed = True
                break
        if placed:
            continue
        # split across bins with most free space first
        while n > 0:
            bi = max(range(len(caps)), key=lambda b: free[b])
            k = min(n, free[bi] // R)
            if k == 0:
                return None
            segs.append({"bin": bi, "off": caps[bi] - free[bi],
                         "n": k, "R": R, "mxoff": mxoff})
            free[bi] -= k * R
            n -= k
            mxoff += k
    return segs


def _plan_key(p):
    return (p.NT_h[0], p.NT_h[1], p.nbins, p.NR, tuple(p.bin_used),
            tuple((s["bin"], s["off"], s["n"], s["R"], s["mxoff"], s["eng"])
                  for s in p.segs))


# ---------------------------------------------------------------------------
# device program
# ---------------------------------------------------------------------------

def build_nc(p):
    NT, NT0 = p.NT, p.NT_h[0]
    NBINS, NR, NRT = p.nbins, p.NR, p.NRT

    nc = bacc.Bacc(None, target_bir_lowering=False, debug=False,
                   num_devices=NCORES)

    imr_e = nc.declare_dram_parameter("imr", [NR, D], BF16, isOutput=False)
    st_e = nc.declare_dram_parameter("st", [NT, 128, KC, 128], F8,
                                     isOutput=False)
    gmat_e = nc.declare_dram_parameter("gmat", [128, NT * 128], BF16,
                                       isOutput=False)
    ident_e = nc.declare_dram_parameter("ident", [128, 128], F32,
                                        isOutput=False)
    identbf_e = nc.declare_dram_parameter("identbf", [128, 128], BF16,
                                          isOutput=False)
    pos0_e = nc.declare_dram_parameter("pos0", [128, 32], F32, isOutput=False)
    pos1_e = nc.declare_dram_parameter("pos1", [128, 32], F32, isOutput=False)
    post0_e = nc.declare_dram_parameter("post0", [32, 128], F32, isOutput=False)
    post1_e = nc.declare_dram_parameter("post1", [32, 128], F32, isOutput=False)
    out_e = nc.declare_dram_parameter("out", [128, 6], F32, isOutput=True)

    with tile.TileContext(nc) as tc:
        from contextlib import ExitStack

        with ExitStack() as ctx:
            const = ctx.enter_context(tc.tile_pool(name="const", bufs=1))
            small = ctx.enter_context(tc.tile_pool(name="small", bufs=1))
            stp = ctx.enter_context(tc.tile_pool(name="stp", bufs=3))
            mxp = ctx.enter_context(tc.tile_pool(name="mxp", bufs=NT0 + 4))
            prep = ctx.enter_context(tc.tile_pool(name="prep", bufs=2))
            gsc = ctx.enter_context(tc.tile_pool(name="gsc", bufs=3))
            # PSUM (8 banks): align+gram (NBINS banks) x bufs + S 1 + misc 1
            pal = ctx.enter_context(
                tc.tile_pool(name="pal", bufs=max(2, 6 // NBINS),
                             space="PSUM"))
            pmisc = ctx.enter_context(
                tc.tile_pool(name="pmisc", bufs=1, space="PSUM"))
            psacc = ctx.enter_context(
                tc.tile_pool(name="psacc", bufs=1, space="PSUM"))

            def misc_psum(shape, name):
                return pmisc.tile(shape, F32, tag="misc", bufs=1, name=name)

            # ---- consts ----
            ident = const.tile([128, 128], F32, tag="ident")
            nc.sync.dma_start(out=ident[:, :], in_=ident_e[:, :])
            identbf = const.tile([128, 128], BF16, tag="identbf")
            nc.sync.dma_start(out=identbf[:, :], in_=identbf_e[:, :])

            # ---- im prep: load bf16 rows, l2-normalize, cast fp8, transpose
            imt = const.tile([128, KC * NR], F8, tag="imt")
            imt3 = imt.rearrange("p (k n) -> p k n", k=KC)
            for rt in range(NRT):
                pr = min(128, NR - 128 * rt)
                imr_t = prep.tile([pr, D], BF16, tag="imld", name="imr_t",
                                  bufs=3)
                nc.sync.dma_start(out=imr_t[:, :],
                                  in_=imr_e[128 * rt:128 * rt + pr, :])
                sq = prep.tile([pr, D], BF16, tag="imsq", name="sq")
                ssq = small.tile([pr, 1], F32, tag=f"imssq{rt}", name="ssq")
                nc.scalar.activation(sq[:, :], imr_t[:, :], ACT.Square,
                                     accum_out=ssq[:, :])
                # nrm16 = ||row|| / 16 ; rcp = 16 / ||row||
                nrm16 = small.tile([pr, 1], F32, tag=f"imnrm{rt}", name="nrm")
                nc.scalar.activation(nrm16[:, :], ssq[:, :], ACT.Sqrt,
                                     scale=1.0 / 256.0)
                nc.vector.tensor_scalar_max(nrm16[:, :], nrm16[:, :], EPS)
                rcp = small.tile([pr, 1], F32, tag=f"imrcp{rt}", name="rcp")
                nc.vector.reciprocal(rcp[:, :], nrm16[:, :])
                ims_bf = prep.tile([pr, D], BF16, tag="imsbf", name="ims_bf",
                                   bufs=3)
                nc.vector.tensor_scalar(
                    out=ims_bf[:, :], in0=imr_t[:, :], scalar1=rcp[:, :],
                    scalar2=None, op0=ALU.mult,
                )
                tr_ps = pmisc.tile([128, KC * pr], BF16, tag="misc", bufs=1,
                                   name="tr_ps")
                for k in range(KC):
                    nc.tensor.transpose(
                        tr_ps[:, pr * k:pr * (k + 1)],
                        ims_bf[:, 128 * k:128 * (k + 1)],
                        identbf[0:pr, 0:pr])
                dst = imt3[:, :, 128 * rt:128 * rt + pr]
                src = tr_ps.rearrange("p (k c) -> p k c", k=KC)
                if rt % 2 == 0:
                    nc.vector.tensor_copy(dst, src)
                else:
                    nc.scalar.copy(dst, src)

            # ---- epilogue consts/buffers ----
            gmat = const.tile([128, NT * 128], BF16, tag="gmat")
            nc.sync.dma_start(out=gmat[:, :], in_=gmat_e[:, :])
            pos0 = const.tile([128, 32], F32, tag="pos0")
            nc.sync.dma_start(out=pos0[:, :], in_=pos0_e[:, :])
            pos1 = const.tile([128, 32], F32, tag="pos1")
            nc.sync.dma_start(out=pos1[:, :], in_=pos1_e[:, :])
            post0 = const.tile([32, 128], F32, tag="post0")
            nc.sync.dma_start(out=post0[:, :], in_=post0_e[:, :])
            post1 = const.tile([32, 128], F32, tag="post1")
            nc.sync.dma_start(out=post1[:, :], in_=post1_e[:, :])
            margin128 = const.tile([128, 1], F32, tag="margin128")
            nc.gpsimd.memset(margin128[:, :], MARGIN)

            posm = [pos0, pos1]
            payload = small.tile([128, 6], F32, tag="payload")
            snd = [small.tile([128, 32], F32, tag=f"snd{h}", name=f"snd{h}")
                   for h in range(2)]
            trash = small.tile([128, 128], BF16, tag="trash")
            trash32 = small.tile([128, 32], F32, tag="trash32")
            negm = [small.tile([128, 32], F32, tag=f"negm{h}", name=f"negm{h}")
                    for h in range(2)]
            nc.vector.tensor_scalar_mul(negm[0][:, :], pos0[:, :], NEG)
            nc.vector.tensor_scalar_mul(negm[1][:, :], pos1[:, :], NEG)
            posr = [small.tile([128, 32], F32R, tag=f"posr{h}", name=f"posr{h}")
                    for h in range(2)]
            nc.scalar.copy(posr[0][:, :], pos0[:, :])
            nc.scalar.copy(posr[1][:, :], pos1[:, :])
            postr = [small.tile([32, 128], F32R, tag=f"postr{h}",
                                name=f"postr{h}") for h in range(2)]
            nc.scalar.copy(postr[0][:, :], post0[:, :])
            nc.scalar.copy(postr[1][:, :], post1[:, :])
            rm = small.tile([32, 2], F32, tag="rm")

            # s-norm scratch: sscale_sq[:, t] = ||s_row(p of tile t)||^2
            sscale_sq = small.tile([128, NT], F32, tag="sscale_sq")
            sscale = small.tile([128, NT], F32, tag="sscale")

            # S accumulators: both halves share one PSUM bank
            s_acc = psacc.tile([128, 64], F32, tag="S", name="S")
            s_ps = [s_acc[:, 0:32], s_acc[:, 32:64]]

            mx_tiles = {}

            def emit_tile(t):
                st_t = stp.tile([128, KC * 128], F8, tag="st")
                nc.sync.dma_start(
                    out=st_t.rearrange("p (k c) -> p k c", k=KC),
                    in_=st_e[t, :, :, :],
                )
                st3 = st_t.rearrange("p (k c) -> p k c", k=KC)
                # single PSUM accumulation tensor (NBINS banks): align bins
                # in cols [0, used_bi); gram shares the last bank at 384:512.
                # One tensor keeps the scheduler in k-major emission order so
                # one weight load serves 1+NBINS matmuls after LDW dedup.
                ps_t = pal.tile([128, NBINS * 512], F32, tag="al",
                                name="ps")
                psv = ps_t.rearrange("p (c n) -> p c n", c=NBINS)
                gram = psv[:, NBINS - 1, 384:512]
                ps = [psv[:, bi, 0:p.bin_used[bi]] for bi in range(NBINS)]
                for kp in range(KC // 2):
                    w = st3[:, 2 * kp:2 * kp + 2, :]
                    for bi in range(NBINS):
                        nc.tensor.matmul(
                            ps[bi],
                            lhsT=w,
                            rhs=imt3[:, 2 * kp:2 * kp + 2,
                                     p.bin_off[bi]:p.bin_off[bi]
                                     + p.bin_used[bi]],
                            start=(kp == 0), stop=(kp == KC // 2 - 1),
                            perf_mode=DR, skip_group_check=True,
                        )
                    # gram shares the last bin's bank: PSUM start zeroing is
                    # bank-granular (2KB), so the bin's kp0 start already
                    # marks gram's byte range pending-zero; gram must never
                    # issue its own start or it would re-mark the bank and
                    # drop the bin's kp0 contribution.
                    nc.tensor.matmul(
                        gram, lhsT=w, rhs=w,
                        start=False, stop=(kp == KC // 2 - 1),
                        perf_mode=DR, skip_group_check=True,
                    )
                # max over image rows -> mx [128, 32]
                mx = mxp.tile([128, 32], F32, tag="mx", name="mx")
                for s in p.segs:
                    w = s["n"] * s["R"]
                    src = psv[:, s["bin"], s["off"]:s["off"] + w]
                    if s["eng"] == "dve":
                        nc.vector.tensor_reduce(
                            out=mx[:, s["mxoff"]:s["mxoff"] + s["n"]],
                            in_=src.rearrange("p (n r) -> p n r", r=s["R"]),
                            axis=AX.X, op=ALU.max,
                        )
                    else:
                        cp = gsc.tile([128, w], BF16, tag=f"gsc{s['mxoff']}",
                                      name="cp")
                        nc.scalar.copy(cp[:, :], src)
                        nc.gpsimd.tensor_reduce(
                            out=mx[:, s["mxoff"]:s["mxoff"] + s["n"]],
                            in_=cp.rearrange("p (n r) -> p n r", r=s["R"]),
                            axis=AX.X, op=ALU.max,
                        )
                # gram diag -> sscale_sq[:, t] (after the segment reduces so
                # the in-order DVE never blocks a prior tile's reduces on
                # this tile's matmuls)
                nc.vector.scalar_tensor_tensor(
                    out=trash[:, :], in0=gram, scalar=1.0,
                    in1=ident[:, :], op0=ALU.mult, op1=ALU.mult,
                    accum_out=sscale_sq[:, t:t + 1],
                )
                mx_tiles[t] = mx

            def emit_sscale_batch(t0, n):
                # sscale cols: 1 / (16 * sqrt(q)) = 1 / sqrt(256 q)
                nc.scalar.activation(sscale[:, t0:t0 + n],
                                     sscale_sq[:, t0:t0 + n],
                                     ACT.Sqrt, scale=256.0)
                nc.vector.tensor_scalar_max(sscale[:, t0:t0 + n],
                                            sscale[:, t0:t0 + n], EPS)
                nc.vector.reciprocal(sscale[:, t0:t0 + n],
                                     sscale[:, t0:t0 + n])

            def emit_g_tile(t):
                mx_r = small.tile([128, 32], BF16, tag="mx_r",
                                  name="mx_r", bufs=4)
                nc.scalar.mul(mx_r[:, :], mx_tiles[t][:, :],
                              mul=sscale[:, t:t + 1])
                h = 0 if t < NT0 else 1
                t0 = 0 if h == 0 else NT0
                nc.tensor.matmul(
                    s_ps[h],
                    lhsT=gmat[:, 128 * t:128 * (t + 1)],
                    rhs=mx_r[:, :],
                    start=(t == t0), stop=(t == t0 + p.NT_h[h] - 1),
                )

            def emit_stats_h(h):
                # diag extraction: accum_out = sum(S * pos) -> payload col 2+h
                nc.vector.scalar_tensor_tensor(
                    out=trash32[:, :], in0=s_ps[h], scalar=1.0,
                    in1=posm[h][:, :], op0=ALU.mult, op1=ALU.mult,
                    accum_out=payload[:, 2 + h:3 + h],
                )
                nc.vector.tensor_add(snd[h][:, :], s_ps[h], negm[h][:, :])
                nc.vector.tensor_reduce(out=payload[:, h:h + 1],
                                        in_=snd[h][:, :], axis=AX.X,
                                        op=ALU.max)
                stp_ps = misc_psum([32, 128], "stp_ps")
                nc.tensor.transpose(stp_ps[:, :], snd[h][:, :], ident[:, :])
                nc.vector.tensor_reduce(out=rm[:, h:h + 1], in_=stp_ps[:, :],
                                        axis=AX.X, op=ALU.max)

            # ---- main loop: sscale in batches of 6, G drained lag 7 ----
            GLAG, SB = 7, 6
            for t in range(NT):
                emit_tile(t)
                if t % SB == SB - 1:
                    emit_sscale_batch(t - SB + 1, SB)
                if t - GLAG >= 0:
                    emit_g_tile(t - GLAG)
                if t - GLAG == NT0 - 1:
                    emit_stats_h(0)
            if NT % SB != 0:
                emit_sscale_batch(NT - NT % SB, NT % SB)
            for t in range(max(0, NT - GLAG), NT):
                emit_g_tile(t)
            emit_stats_h(1)

            # ---- row-hinge epilogue ----
            rowmax = small.tile([32, 1], F32, tag="rowmax")
            nc.vector.tensor_max(rowmax[:, :], rm[:, 0:1], rm[:, 1:2])
            # own-diag per image (row order): for each half h, pos_h^T @ d_h
            dca = small.tile([128, 2], F32R, tag="dca")
            dcb = small.tile([128, 2], F32R, tag="dcb")
            nc.scalar.copy(dca[:, 0:1], payload[:, 2:3])
            nc.scalar.mul(dca[:, 1:2], payload[:, 2:3], mul=0.0)
            nc.scalar.copy(dcb[:, 0:1], payload[:, 3:4])
            nc.scalar.mul(dcb[:, 1:2], payload[:, 3:4], mul=0.0)
            dfree_ps = misc_psum([32, 2], "dfree_ps")
            nc.tensor.matmul(dfree_ps[:, :], lhsT=posr[0][:, :],
                             rhs=dca[:, :], start=True, stop=False)
            nc.tensor.matmul(dfree_ps[:, :], lhsT=posr[1][:, :],
                             rhs=dcb[:, :], start=False, stop=True)
            dfree_sb = small.tile([32, 1], F32, tag="dfree_sb")
            nc.scalar.copy(dfree_sb[:, :], dfree_ps[:, 0:1])
            rh_pre = small.tile([32, 2], F32, tag="rh_pre")
            nc.gpsimd.memset(rh_pre[:, :], 0.0)
            nc.vector.tensor_sub(rh_pre[:, 0:1], rowmax[:, :], dfree_sb[:, :])
            rowhinge = small.tile([32, 2], F32R, tag="rowhinge")
            nc.scalar.activation(rowhinge[:, :], rh_pre[:, :], ACT.Relu,
                                 bias=margin128[0:32, :])
            for h in range(2):
                rh_ps = misc_psum([128, 2], "rh_ps")
                nc.tensor.matmul(rh_ps[:, :], lhsT=postr[h][:, :],
                                 rhs=rowhinge[:, :], start=True, stop=True)
                nc.scalar.copy(payload[:, 4 + h:5 + h], rh_ps[:, 0:1])

            nc.sync.dma_start(out=out_e[:, :], in_=payload[:, :])

    nc.finalize()
    return nc


# ---------------------------------------------------------------------------
# host side
# ---------------------------------------------------------------------------

def build_in_maps(p, im_set, s_seq):
    im_set = np.asarray(im_set, dtype=np.float32)
    s_seq = np.asarray(s_seq, dtype=np.float32)
    NT, NT0, NR = p.NT, p.NT_h[0], p.NR

    # s tiles (shared): fp8 of raw word rows in compacted order
    s8 = np.zeros((NT * 128, D), dtype=np.float32)
    gmat = np.zeros((128, NT * 128), dtype=np.float32)
    for h in (0, 1):
        base = 0 if h == 0 else NT0 * 128
        for i, cj in enumerate(p.srows[h]):
            if cj is None:
                continue
            c, j = cj
            s8[base + i] = s_seq[c, 1 + j]
            t, pp = divmod(base + i, 128)
            gmat[pp, 128 * t + (c - 128 * h)] = 1.0
    s8 = np.clip(s8, -240.0, 240.0).astype(ml_dtypes.float8_e4m3)
    gmat = gmat.astype(ml_dtypes.bfloat16)
    st = np.ascontiguousarray(
        s8.reshape(NT, 128, KC, 128).transpose(0, 3, 2, 1))

    ident = np.eye(128, dtype=np.float32)
    identbf = ident.astype(ml_dtypes.bfloat16)

    in_maps = []
    for m in range(NCORES):
        imr = np.zeros((NR, D), dtype=np.float32)
        pos0 = np.zeros((128, 32), np.float32)
        pos1 = np.zeros((128, 32), np.float32)
        for i in range(32):
            b = int(p.order[8 * i + m])
            off = p.slot_off[i]
            nvalid = int(p.im_l[b])
            imr[off:off + nvalid] = im_set[b, 1:1 + nvalid]
            if b < 128:
                pos0[b % 128, i] = 1.0
            else:
                pos1[b % 128, i] = 1.0
        in_maps.append({
            "imr": imr.astype(ml_dtypes.bfloat16),
            "st": st,
            "gmat": gmat,
            "ident": ident,
            "identbf": identbf,
            "pos0": pos0,
            "pos1": pos1,
            "post0": np.ascontiguousarray(pos0.T),
            "post1": np.ascontiguousarray(pos1.T),
        })
    return in_maps


def host_combine(outs):
    """Combine the 8 cores' [128, 6] payloads into the scalar loss."""
    agg = np.stack([np.asarray(o, dtype=np.float32) for o in outs])  # [8,128,6]
    colmax = agg[:, :, 0:2].max(axis=0)          # [128, 2]
    diag = agg[:, :, 2:4].sum(axis=0)            # [128, 2]
    colhinge = np.maximum(MARGIN + colmax - diag, 0.0).sum()
    rowhinge = agg[:, :, 4:6].sum()
    return np.float32(colhinge + rowhinge)


_NC_CACHE = {}


def kernel(im_set, s_seq, im_len, s_len):
    global LAST_RESULT
    im_len = np.asarray(im_len, dtype=np.int32)
    s_len = np.asarray(s_len, dtype=np.int32)
    im_l = im_len - 1
    s_l = s_len - 3

    p = plan_layout(im_l, s_l)
    p.im_l = im_l
    key = _plan_key(p)
    if key not in _NC_CACHE:
        nc = build_nc(p)
        if LDW_DEDUP:
            _orig = nc.to_json_bytes

            def _to_json_bytes_dedup(_orig=_orig):
                js, _ = _dedup_ldweights_json(_orig())
                return js

            nc.to_json_bytes = _to_json_bytes_dedup
        _NC_CACHE[key] = nc
    nc = _NC_CACHE[key]

    in_maps = build_in_maps(p, im_set, s_seq)
    res = run_bass_kernel_spmd(nc, in_maps, core_ids=list(range(NCORES)))
    LAST_RESULT = res
    return host_combine([r["out"] for r in res.results])


# revision 26
# speedup vs baseline: 1.1778x; 1.1778x over previous
"""Distributed Trainium2 Bass kernel for AlignmentContrastiveLoss (v2).

Reference computation (B=256, L_im=37, L_s=33, D=1024):
    im  = l2norm(im_set)[:, 1:, :]   masked by im_len-1     [B, 36, D]
    s   = l2norm(s_seq)[:, 1:-2, :]  masked by s_len-3      [B, 30, D]
    align[b,c,i,j] = im[b,i] . s[c,j]   (masked entries -> 0)
    scores[b,c] = sum_j max_i align[b,c,i,j]
    loss = sum_b relu(M + max_{c!=b} scores[b,c] - scores[b,b])
         + sum_c relu(M + max_{b!=c} scores[b,c] - scores[c,c])

v2 strategy (vs the bf16 full-density baseline):
  * s side is compacted on the host: only the valid (c, j) word rows are
    shipped (plus zero padding to 128-row tiles, each 128-sentence half
    padded separately so every tile maps to one half).  NT drops 60 -> ~36.
  * im side: each image's valid rows are padded up to R in {12,...,36}
    (multiple of G=6, with >=1 zero row unless im_l==36 so the reference's
    max-includes-zero clamp is preserved).  Images are rank-sorted by R and
    dealt round-robin to the 8 cores, so all cores share one R "template"
    (SPMD requires identical reduce shapes); rows ~9216 -> ~5300+pad/core... 888.
  * The big einsum runs in fp8 e4m3 with DoubleRow perf mode (K=256 per
    instruction at 0.5 cycles/row): ~4x fewer PE cycles than bf16.
    s is quantized host-side (raw values, |s| << 240); im is normalized on
    device, scaled x16 and cast to fp8; the 1/16 folds into the s scale.
  * No collectives at all: s norms come from the diagonal of per-tile
    fp8 Gram matmuls on the PE (each core loads all s tiles anyway), and
    the final cross-core combine (max/sum over 8 cores' 128x6 payloads)
    happens on the host - that is the gather/unshard step.
  * The max-over-i reduces are split between DVE (direct PSUM reads) and
    GpSimd (via a ScalarE PSUM->SBUF bf16 copy), since DVE alone would be
    the bottleneck at ~1.12 ns/elem.

The device program shape depends only on (im_len, s_len); build_nc is
cached on those layout parameters and recompiled if they change.
"""

import os
import sys

import numpy as np
import ml_dtypes

for _p in ("/opt/trn_rl_repo", "/root/.axon_site/_ro/trn_rl_repo"):
    if os.path.isdir(_p) and _p not in sys.path:
        sys.path.append(_p)

import concourse.bass as bass
import concourse.mybir as mybir
import concourse.tile as tile
from concourse import bacc
from concourse.bass_utils import run_bass_kernel_spmd


def _ensure_axon_hooks():
    """Some agent images ship an ``antenv`` without ``axon_hooks``, but
    bass_utils hard-imports it when trace=True.  Provide the registry and,
    when libaxon_pjrt.so is available, the real NTFF profile hook."""
    import types

    try:
        import antenv.axon_hooks  # noqa: F401
        return
    except ImportError:
        pass
    try:
        import antenv
    except ImportError:
        return
    mod = types.ModuleType("antenv.axon_hooks")
    mod._hook = None
    mod.set_axon_ntff_profile_hook = lambda h: setattr(mod, "_hook", h)
    mod.get_axon_ntff_profile_hook = lambda: mod._hook
    sys.modules["antenv.axon_hooks"] = mod
    antenv.axon_hooks = mod
    so_path = "/opt/axon/libaxon_pjrt.so"
    try:
        import trn_agent_boot.trn_boot as _tb
        if os.path.exists(so_path):
            mod._hook = _tb._ntff_profile_via_ctypes(so_path)
    except Exception:
        pass


_ensure_axon_hooks()

F32 = mybir.dt.float32
F32R = mybir.dt.float32r
BF16 = mybir.dt.bfloat16
F8 = mybir.dt.float8e4
I32 = mybir.dt.int32
AX = mybir.AxisListType
ALU = mybir.AluOpType
ACT = mybir.ActivationFunctionType
DR = mybir.MatmulPerfMode.DoubleRow

NCORES = 8
B, LI, LS, D = 256, 36, 30, 1024
KC = D // 128               # 8 contraction chunks of 128
G = 6                       # im row-padding granularity
MARGIN, EPS, NEG = 0.2, 1e-12, -1.0e9

LAST_RESULT = None  # BassKernelResults of the most recent run (for test harness)

# Dedup redundant PE weight loads: bass lowering splits every matmul into a
# standalone Ldweights + non-self-loading Matmult, but emits one Ldweights
# per matmul even when consecutive matmuls share the same stationary
# operand.  We post-process the BIR json and drop a generated Ldweights
# (no semaphore waits/updates) when the weights signature matches what the
# PE already has loaded.  This is the dominant PE cost for DoubleRow fp8
# (256-column loads at ~112 ns apiece).
LDW_DEDUP = os.environ.get("LDW_DEDUP", "1") == "1"


def _dedup_ldweights_json(js_bytes):
    import json as _json

    j = _json.loads(js_bytes)
    dropped = 0
    for fn in j.get("functions", []):
        for blk in fn.get("blocks", []):
            insts = blk.get("instructions")
            if not insts:
                continue
            out = []
            loaded = None
            for x in insts:
                if x.get("engine") != "PE":
                    out.append(x)
                    continue
                op = x.get("opcode")
                if op == "Ldweights":
                    sig = _json.dumps(
                        [x.get("ins"), x.get("perf_mode"),
                         x.get("tile_size"), x.get("tile_position"),
                         x.get("is_transpose")], sort_keys=True)
                    sync = x.get("sync_info") or {}
                    if (sig == loaded and not sync.get("on_wait")
                            and not sync.get("on_update")):
                        dropped += 1
                        continue
                    loaded = sig
                    out.append(x)
                elif op == "Matmult":
                    if x.get("ldweights") is not False:
                        loaded = None  # self-loading matmul clobbers weights
                    out.append(x)
                else:
                    loaded = None
                    out.append(x)
            blk["instructions"] = out
    return _json.dumps(j).encode(), dropped


# ---------------------------------------------------------------------------
# layout planning (data-dependent, host side)
# ---------------------------------------------------------------------------

class Plan:
    pass


def plan_layout(im_l, s_l):
    p = Plan()
    # ---- s side: per-half compacted row lists ----
    p.NT_h = []
    p.srows = []            # per half: list of (c, j) or None (pad)
    for h in (0, 1):
        rows = [(c, j) for c in range(128 * h, 128 * h + 128)
                for j in range(int(s_l[c]))]
        nt = -(-len(rows) // 128)
        rows = rows + [None] * (nt * 128 - len(rows))
        p.NT_h.append(nt)
        p.srows.append(rows)
    p.NT = p.NT_h[0] + p.NT_h[1]

    # ---- im side: R template shared across cores ----
    R = np.where(im_l >= LI, LI,
                 (G * np.ceil((im_l + 1) / G)).astype(np.int64)).astype(np.int64)
    order = np.argsort(-R, kind="stable")
    p.order = order                       # slot i of core m -> image order[8i+m]
    p.template = [int(R[order[8 * i]]) for i in range(32)]
    # runs of equal R in the (descending) template
    runs = []
    i = 0
    while i < 32:
        j = i
        while j < 32 and p.template[j] == p.template[i]:
            j += 1
        runs.append({"mxoff": i, "n": j - i, "R": p.template[i]})
        i = j
    # pack runs into PSUM banks; the LAST bin reserves 128 cols for the
    # gram matmul (cols 384:512) so the whole per-tile accumulation
    # tensor is nbins banks.  Splitting a run costs one extra segment.
    for nbins in range(2, 9):
        caps = [512] * (nbins - 1) + [384]
        segs = _pack_runs(runs, caps)
        if segs is not None:
            break
    p.nbins = nbins
    p.segs = segs
    for s in segs:
        s["eng"] = "dve"
    p.NR = sum(s["n"] * s["R"] for s in segs)
    p.NRT = -(-p.NR // 128)
    # im slot row offsets: global row index = bin*512... no - rows are
    # packed per-bin contiguously in imt column space: bin bi occupies
    # imt cols [binoff[bi], binoff[bi]+used[bi])
    used = [0] * nbins
    for s in segs:
        used[s["bin"]] = max(used[s["bin"]], s["off"] + s["n"] * s["R"])
    binoff = [0] * nbins
    for bi in range(1, nbins):
        binoff[bi] = binoff[bi - 1] + used[bi - 1]
    p.bin_used, p.bin_off = used, binoff
    p.slot_off = [0] * 32
    for s in segs:
        for t in range(s["n"]):
            p.slot_off[s["mxoff"] + t] = (binoff[s["bin"]] + s["off"]
                                          + t * s["R"])
    return p


def _pack_runs(runs, caps):
    """First-fit-decreasing of template runs into bins with capacities
    ``caps``; runs may split at image boundaries.  Returns segment list
    or None if it does not fit."""
    free = list(caps)
    segs = []
    for r in sorted(runs, key=lambda r: -r["n"] * r["R"]):
        n, R, mxoff = r["n"], r["R"], r["mxoff"]
        # whole-run first fit
        placed = False
        for bi in range(len(caps)):
            if n * R <= free[bi]:
                segs.append({"bin": bi, "off": caps[bi] - free[bi],
                             "n": n, "R": R, "mxoff": mxoff})
                free[bi] -= n * R
                placed = True
                break
        if placed:
            continue
        # split across bins with most free space first
        while n > 0:
            bi = max(range(len(caps)), key=lambda b: free[b])
            k = min(n, free[bi] // R)
            if k == 0:
                return None
            segs.append({"bin": bi, "off": caps[bi] - free[bi],
                         "n": k, "R": R, "mxoff": mxoff})
            free[bi] -= k * R
            n -= k
            mxoff += k
    return segs


def _plan_key(p):
    return (p.NT_h[0], p.NT_h[1], p.nbins, p.NR, tuple(p.bin_used),
            tuple((s["bin"], s["off"], s["n"], s["R"], s["mxoff"], s["eng"])
                  for s in p.segs))


# ---------------------------------------------------------------------------
# device program
# ---------------------------------------------------------------------------

def build_nc(p):
    NT, NT0 = p.NT, p.NT_h[0]
    NBINS, NR, NRT = p.nbins, p.NR, p.NRT

    nc = bacc.Bacc(None, target_bir_lowering=False, debug=False,
                   num_devices=NCORES)

    imr_e = nc.declare_dram_parameter("imr", [NR, D], BF16, isOutput=False)
    st_e = nc.declare_dram_parameter("st", [NT, 128, KC, 128], F8,
                                     isOutput=False)
    gmat_e = nc.declare_dram_parameter("gmat", [128, NT * 128], BF16,
                                       isOutput=False)
    ident_e = nc.declare_dram_parameter("ident", [128, 128], F32,
                                        isOutput=False)
    identbf_e = nc.declare_dram_parameter("identbf", [128, 128], BF16,
                                          isOutput=False)
    pos0_e = nc.declare_dram_parameter("pos0", [128, 32], F32, isOutput=False)
    pos1_e = nc.declare_dram_parameter("pos1", [128, 32], F32, isOutput=False)
    post0_e = nc.declare_dram_parameter("post0", [32, 128], F32, isOutput=False)
    post1_e = nc.declare_dram_parameter("post1", [32, 128], F32, isOutput=False)
    out_e = nc.declare_dram_parameter("out", [128, 6], F32, isOutput=True)

    with tile.TileContext(nc) as tc:
        from contextlib import ExitStack

        with ExitStack() as ctx:
            const = ctx.enter_context(tc.tile_pool(name="const", bufs=1))
            small = ctx.enter_context(tc.tile_pool(name="small", bufs=1))
            stp = ctx.enter_context(tc.tile_pool(name="stp", bufs=3))
            mxp = ctx.enter_context(tc.tile_pool(name="mxp", bufs=NT0 + 4))
            prep = ctx.enter_context(tc.tile_pool(name="prep", bufs=2))
            gsc = ctx.enter_context(tc.tile_pool(name="gsc", bufs=3))
            # PSUM (8 banks): align+gram (NBINS banks) x bufs + S 1 + misc 1
            pal = ctx.enter_context(
                tc.tile_pool(name="pal", bufs=max(2, 6 // NBINS),
                             space="PSUM"))
            pmisc = ctx.enter_context(
                tc.tile_pool(name="pmisc", bufs=1, space="PSUM"))
            psacc = ctx.enter_context(
                tc.tile_pool(name="psacc", bufs=1, space="PSUM"))

            def misc_psum(shape, name):
                return pmisc.tile(shape, F32, tag="misc", bufs=1, name=name)

            # ---- consts ----
            ident = const.tile([128, 128], F32, tag="ident")
            nc.sync.dma_start(out=ident[:, :], in_=ident_e[:, :])
            identbf = const.tile([128, 128], BF16, tag="identbf")
            nc.sync.dma_start(out=identbf[:, :], in_=identbf_e[:, :])

            # ---- im prep: load bf16 rows, l2-normalize, cast fp8, transpose
            imt = const.tile([128, KC * NR], F8, tag="imt")
            imt3 = imt.rearrange("p (k n) -> p k n", k=KC)
            for rt in range(NRT):
                pr = min(128, NR - 128 * rt)
                imr_t = prep.tile([pr, D], BF16, tag="imld", name="imr_t",
                                  bufs=3)
                nc.sync.dma_start(out=imr_t[:, :],
                                  in_=imr_e[128 * rt:128 * rt + pr, :])
                sq = prep.tile([pr, D], BF16, tag="imsq", name="sq")
                ssq = small.tile([pr, 1], F32, tag=f"imssq{rt}", name="ssq")
                nc.scalar.activation(sq[:, :], imr_t[:, :], ACT.Square,
                                     accum_out=ssq[:, :])
                # nrm16 = ||row|| / 16 ; rcp = 16 / ||row||
                nrm16 = small.tile([pr, 1], F32, tag=f"imnrm{rt}", name="nrm")
                nc.scalar.activation(nrm16[:, :], ssq[:, :], ACT.Sqrt,
                                     scale=1.0 / 256.0)
                nc.vector.tensor_scalar_max(nrm16[:, :], nrm16[:, :], EPS)
                rcp = small.tile([pr, 1], F32, tag=f"imrcp{rt}", name="rcp")
                nc.vector.reciprocal(rcp[:, :], nrm16[:, :])
                ims_bf = prep.tile([pr, D], BF16, tag="imsbf", name="ims_bf",
                                   bufs=3)
                nc.vector.tensor_scalar(
                    out=ims_bf[:, :], in0=imr_t[:, :], scalar1=rcp[:, :],
                    scalar2=None, op0=ALU.mult,
                )
                tr_ps = pmisc.tile([128, KC * pr], BF16, tag="misc", bufs=1,
                                   name="tr_ps")
                for k in range(KC):
                    nc.tensor.transpose(
                        tr_ps[:, pr * k:pr * (k + 1)],
                        ims_bf[:, 128 * k:128 * (k + 1)],
                        identbf[0:pr, 0:pr])
                dst = imt3[:, :, 128 * rt:128 * rt + pr]
                src = tr_ps.rearrange("p (k c) -> p k c", k=KC)
                nc.scalar.copy(dst, src)

            # ---- epilogue consts/buffers ----
            gmat = const.tile([128, NT * 128], BF16, tag="gmat")
            nc.sync.dma_start(out=gmat[:, :], in_=gmat_e[:, :])
            pos0 = const.tile([128, 32], F32, tag="pos0")
            nc.sync.dma_start(out=pos0[:, :], in_=pos0_e[:, :])
            pos1 = const.tile([128, 32], F32, tag="pos1")
            nc.sync.dma_start(out=pos1[:, :], in_=pos1_e[:, :])
            post0 = const.tile([32, 128], F32, tag="post0")
            nc.sync.dma_start(out=post0[:, :], in_=post0_e[:, :])
            post1 = const.tile([32, 128], F32, tag="post1")
            nc.sync.dma_start(out=post1[:, :], in_=post1_e[:, :])
            margin128 = const.tile([128, 1], F32, tag="margin128")
            nc.gpsimd.memset(margin128[:, :], MARGIN)

            posm = [pos0, pos1]
            payload = small.tile([128, 6], F32, tag="payload")
            snd = [small.tile([128, 32], F32, tag=f"snd{h}", name=f"snd{h}")
                   for h in range(2)]
            trash = small.tile([128, 128], BF16, tag="trash")
            trash32 = small.tile([128, 32], F32, tag="trash32")
            negm = [small.tile([128, 32], F32, tag=f"negm{h}", name=f"negm{h}")
                    for h in range(2)]
            nc.vector.tensor_scalar_mul(negm[0][:, :], pos0[:, :], NEG)
            nc.vector.tensor_scalar_mul(negm[1][:, :], pos1[:, :], NEG)
            posr = [small.tile([128, 32], F32R, tag=f"posr{h}", name=f"posr{h}")
                    for h in range(2)]
            nc.scalar.copy(posr[0][:, :], pos0[:, :])
            nc.scalar.copy(posr[1][:, :], pos1[:, :])
            postr = [small.tile([32, 128], F32R, tag=f"postr{h}",
                                name=f"postr{h}") for h in range(2)]
            nc.scalar.copy(postr[0][:, :], post0[:, :])
            nc.scalar.copy(postr[1][:, :], post1[:, :])
            rm = small.tile([32, 2], F32, tag="rm")

            # s-norm scratch: sscale_sq[:, t] = ||s_row(p of tile t)||^2
            sscale_sq = small.tile([128, NT], F32, tag="sscale_sq")
            sscale = small.tile([128, NT], F32, tag="sscale")

            # S accumulators: both halves share one PSUM bank
            s_acc = psacc.tile([128, 64], F32, tag="S", name="S")
            s_ps = [s_acc[:, 0:32], s_acc[:, 32:64]]

            mx_tiles = {}

            def emit_tile(t):
                st_t = stp.tile([128, KC * 128], F8, tag="st")
                nc.sync.dma_start(
                    out=st_t.rearrange("p (k c) -> p k c", k=KC),
                    in_=st_e[t, :, :, :],
                )
                st3 = st_t.rearrange("p (k c) -> p k c", k=KC)
                # single PSUM accumulation tensor (NBINS banks): align bins
                # in cols [0, used_bi); gram shares the last bank at 384:512.
                # One tensor keeps the scheduler in k-major emission order so
                # one weight load serves 1+NBINS matmuls after LDW dedup.
                ps_t = pal.tile([128, NBINS * 512], F32, tag="al",
                                name="ps")
                psv = ps_t.rearrange("p (c n) -> p c n", c=NBINS)
                gram = psv[:, NBINS - 1, 384:512]
                ps = [psv[:, bi, 0:p.bin_used[bi]] for bi in range(NBINS)]
                for kp in range(KC // 2):
                    w = st3[:, 2 * kp:2 * kp + 2, :]
                    for bi in range(NBINS):
                        nc.tensor.matmul(
                            ps[bi],
                            lhsT=w,
                            rhs=imt3[:, 2 * kp:2 * kp + 2,
                                     p.bin_off[bi]:p.bin_off[bi]
                                     + p.bin_used[bi]],
                            start=(kp == 0), stop=(kp == KC // 2 - 1),
                            perf_mode=DR, skip_group_check=True,
                        )
                    # gram shares the last bin's bank: PSUM start zeroing is
                    # bank-granular (2KB), so the bin's kp0 start already
                    # marks gram's byte range pending-zero; gram must never
                    # issue its own start or it would re-mark the bank and
                    # drop the bin's kp0 contribution.
                    nc.tensor.matmul(
                        gram, lhsT=w, rhs=w,
                        start=False, stop=(kp == KC // 2 - 1),
                        perf_mode=DR, skip_group_check=True,
                    )
                # max over image rows -> mx [128, 32]
                mx = mxp.tile([128, 32], F32, tag="mx", name="mx")
                for s in p.segs:
                    w = s["n"] * s["R"]
                    src = psv[:, s["bin"], s["off"]:s["off"] + w]
                    if s["eng"] == "dve":
                        nc.vector.tensor_reduce(
                            out=mx[:, s["mxoff"]:s["mxoff"] + s["n"]],
                            in_=src.rearrange("p (n r) -> p n r", r=s["R"]),
                            axis=AX.X, op=ALU.max,
                        )
                    else:
                        cp = gsc.tile([128, w], BF16, tag=f"gsc{s['mxoff']}",
                                      name="cp")
                        nc.scalar.copy(cp[:, :], src)
                        nc.gpsimd.tensor_reduce(
                            out=mx[:, s["mxoff"]:s["mxoff"] + s["n"]],
                            in_=cp.rearrange("p (n r) -> p n r", r=s["R"]),
                            axis=AX.X, op=ALU.max,
                        )
                # gram diag -> sscale_sq[:, t] (after the segment reduces so
                # the in-order DVE never blocks a prior tile's reduces on
                # this tile's matmuls)
                nc.vector.scalar_tensor_tensor(
                    out=trash[:, :], in0=gram, scalar=1.0,
                    in1=ident[:, :], op0=ALU.mult, op1=ALU.mult,
                    accum_out=sscale_sq[:, t:t + 1],
                )
                mx_tiles[t] = mx

            def emit_sscale_batch(t0, n):
                # sscale cols: 1 / (16 * sqrt(q)) = 1 / sqrt(256 q)
                nc.scalar.activation(sscale[:, t0:t0 + n],
                                     sscale_sq[:, t0:t0 + n],
                                     ACT.Sqrt, scale=256.0)
                nc.vector.tensor_scalar_max(sscale[:, t0:t0 + n],
                                            sscale[:, t0:t0 + n], EPS)
                nc.vector.reciprocal(sscale[:, t0:t0 + n],
                                     sscale[:, t0:t0 + n])

            def emit_g_tile(t):
                mx_r = small.tile([128, 32], BF16, tag="mx_r",
                                  name="mx_r", bufs=4)
                nc.scalar.mul(mx_r[:, :], mx_tiles[t][:, :],
                              mul=sscale[:, t:t + 1])
                h = 0 if t < NT0 else 1
                t0 = 0 if h == 0 else NT0
                nc.tensor.matmul(
                    s_ps[h],
                    lhsT=gmat[:, 128 * t:128 * (t + 1)],
                    rhs=mx_r[:, :],
                    start=(t == t0), stop=(t == t0 + p.NT_h[h] - 1),
                )

            def emit_stats_h(h):
                # diag extraction: accum_out = sum(S * pos) -> payload col 2+h
                nc.vector.scalar_tensor_tensor(
                    out=trash32[:, :], in0=s_ps[h], scalar=1.0,
                    in1=posm[h][:, :], op0=ALU.mult, op1=ALU.mult,
                    accum_out=payload[:, 2 + h:3 + h],
                )
                nc.vector.tensor_add(snd[h][:, :], s_ps[h], negm[h][:, :])
                nc.vector.tensor_reduce(out=payload[:, h:h + 1],
                                        in_=snd[h][:, :], axis=AX.X,
                                        op=ALU.max)
                stp_ps = misc_psum([32, 128], "stp_ps")
                nc.tensor.transpose(stp_ps[:, :], snd[h][:, :], ident[:, :])
                nc.vector.tensor_reduce(out=rm[:, h:h + 1], in_=stp_ps[:, :],
                                        axis=AX.X, op=ALU.max)

            # ---- main loop: sscale in lagged batches of 6, G drained
            # with lag 8 so no ACT/PE op ever waits on the current tile's
            # DVE output ----
            GLAG, SB, BLAG = 8, 6, 2
            done = [0]

            def drain(upto):
                # emit any complete sscale batch ending at or before `upto`
                while done[0] + SB <= upto + 1:
                    emit_sscale_batch(done[0], SB)
                    done[0] += SB
            for t in range(NT):
                emit_tile(t)
                if t - BLAG >= 0:
                    drain(t - BLAG)
                if t - GLAG >= 0:
                    emit_g_tile(t - GLAG)
                if t - GLAG == NT0 - 1:
                    emit_stats_h(0)
            if done[0] < NT:
                emit_sscale_batch(done[0], NT - done[0])
            for t in range(max(0, NT - GLAG), NT):
                emit_g_tile(t)
            emit_stats_h(1)

            # ---- row-hinge epilogue ----
            rowmax = small.tile([32, 1], F32, tag="rowmax")
            nc.vector.tensor_max(rowmax[:, :], rm[:, 0:1], rm[:, 1:2])
            # own-diag per image (row order): for each half h, pos_h^T @ d_h
            dca = small.tile([128, 2], F32R, tag="dca")
            dcb = small.tile([128, 2], F32R, tag="dcb")
            nc.scalar.copy(dca[:, 0:1], payload[:, 2:3])
            nc.scalar.mul(dca[:, 1:2], payload[:, 2:3], mul=0.0)
            nc.scalar.copy(dcb[:, 0:1], payload[:, 3:4])
            nc.scalar.mul(dcb[:, 1:2], payload[:, 3:4], mul=0.0)
            dfree_ps = misc_psum([32, 2], "dfree_ps")
            nc.tensor.matmul(dfree_ps[:, :], lhsT=posr[0][:, :],
                             rhs=dca[:, :], start=True, stop=False)
            nc.tensor.matmul(dfree_ps[:, :], lhsT=posr[1][:, :],
                             rhs=dcb[:, :], start=False, stop=True)
            dfree_sb = small.tile([32, 1], F32, tag="dfree_sb")
            nc.scalar.copy(dfree_sb[:, :], dfree_ps[:, 0:1])
            rh_pre = small.tile([32, 2], F32, tag="rh_pre")
            nc.gpsimd.memset(rh_pre[:, :], 0.0)
            nc.vector.tensor_sub(rh_pre[:, 0:1], rowmax[:, :], dfree_sb[:, :])
            rowhinge = small.tile([32, 2], F32R, tag="rowhinge")
            nc.scalar.activation(rowhinge[:, :], rh_pre[:, :], ACT.Relu,
                                 bias=margin128[0:32, :])
            for h in range(2):
                rh_ps = misc_psum([128, 2], "rh_ps")
                nc.tensor.matmul(rh_ps[:, :], lhsT=postr[h][:, :],
                                 rhs=rowhinge[:, :], start=True, stop=True)
                nc.scalar.copy(payload[:, 4 + h:5 + h], rh_ps[:, 0:1])

            nc.sync.dma_start(out=out_e[:, :], in_=payload[:, :])

    nc.finalize()
    return nc


# ---------------------------------------------------------------------------
# host side
# ---------------------------------------------------------------------------

def build_in_maps(p, im_set, s_seq):
    im_set = np.asarray(im_set, dtype=np.float32)
    s_seq = np.asarray(s_seq, dtype=np.float32)
    NT, NT0, NR = p.NT, p.NT_h[0], p.NR

    # s tiles (shared): fp8 of raw word rows in compacted order
    s8 = np.zeros((NT * 128, D), dtype=np.float32)
    gmat = np.zeros((128, NT * 128), dtype=np.float32)
    for h in (0, 1):
        base = 0 if h == 0 else NT0 * 128
        for i, cj in enumerate(p.srows[h]):
            if cj is None:
                continue
            c, j = cj
            s8[base + i] = s_seq[c, 1 + j]
            t, pp = divmod(base + i, 128)
            gmat[pp, 128 * t + (c - 128 * h)] = 1.0
    s8 = np.clip(s8, -240.0, 240.0).astype(ml_dtypes.float8_e4m3)
    gmat = gmat.astype(ml_dtypes.bfloat16)
    st = np.ascontiguousarray(
        s8.reshape(NT, 128, KC, 128).transpose(0, 3, 2, 1))

    ident = np.eye(128, dtype=np.float32)
    identbf = ident.astype(ml_dtypes.bfloat16)

    in_maps = []
    for m in range(NCORES):
        imr = np.zeros((NR, D), dtype=np.float32)
        pos0 = np.zeros((128, 32), np.float32)
        pos1 = np.zeros((128, 32), np.float32)
        for i in range(32):
            b = int(p.order[8 * i + m])
            off = p.slot_off[i]
            nvalid = int(p.im_l[b])
            imr[off:off + nvalid] = im_set[b, 1:1 + nvalid]
            if b < 128:
                pos0[b % 128, i] = 1.0
            else:
                pos1[b % 128, i] = 1.0
        in_maps.append({
            "imr": imr.astype(ml_dtypes.bfloat16),
            "st": st,
            "gmat": gmat,
            "ident": ident,
            "identbf": identbf,
            "pos0": pos0,
            "pos1": pos1,
            "post0": np.ascontiguousarray(pos0.T),
            "post1": np.ascontiguousarray(pos1.T),
        })
    return in_maps


def host_combine(outs):
    """Combine the 8 cores' [128, 6] payloads into the scalar loss."""
    agg = np.stack([np.asarray(o, dtype=np.float32) for o in outs])  # [8,128,6]
    colmax = agg[:, :, 0:2].max(axis=0)          # [128, 2]
    diag = agg[:, :, 2:4].sum(axis=0)            # [128, 2]
    colhinge = np.maximum(MARGIN + colmax - diag, 0.0).sum()
    rowhinge = agg[:, :, 4:6].sum()
    return np.float32(colhinge + rowhinge)


_NC_CACHE = {}


def kernel(im_set, s_seq, im_len, s_len):
    global LAST_RESULT
    im_len = np.asarray(im_len, dtype=np.int32)
    s_len = np.asarray(s_len, dtype=np.int32)
    im_l = im_len - 1
    s_l = s_len - 3

    p = plan_layout(im_l, s_l)
    p.im_l = im_l
    key = _plan_key(p)
    if key not in _NC_CACHE:
        nc = build_nc(p)
        if LDW_DEDUP:
            _orig = nc.to_json_bytes

            def _to_json_bytes_dedup(_orig=_orig):
                js, _ = _dedup_ldweights_json(_orig())
                return js

            nc.to_json_bytes = _to_json_bytes_dedup
        _NC_CACHE[key] = nc
    nc = _NC_CACHE[key]

    in_maps = build_in_maps(p, im_set, s_seq)
    res = run_bass_kernel_spmd(nc, in_maps, core_ids=list(range(NCORES)))
    LAST_RESULT = res
    return host_combine([r["out"] for r in res.results])


# revision 27
# speedup vs baseline: 1.2691x; 1.0776x over previous
"""Distributed Trainium2 Bass kernel for AlignmentContrastiveLoss (v2).

Reference computation (B=256, L_im=37, L_s=33, D=1024):
    im  = l2norm(im_set)[:, 1:, :]   masked by im_len-1     [B, 36, D]
    s   = l2norm(s_seq)[:, 1:-2, :]  masked by s_len-3      [B, 30, D]
    align[b,c,i,j] = im[b,i] . s[c,j]   (masked entries -> 0)
    scores[b,c] = sum_j max_i align[b,c,i,j]
    loss = sum_b relu(M + max_{c!=b} scores[b,c] - scores[b,b])
         + sum_c relu(M + max_{b!=c} scores[b,c] - scores[c,c])

v2 strategy (vs the bf16 full-density baseline):
  * s side is compacted on the host: only the valid (c, j) word rows are
    shipped (plus zero padding to 128-row tiles, each 128-sentence half
    padded separately so every tile maps to one half).  NT drops 60 -> ~36.
  * im side: each image's valid rows are padded up to R in {12,...,36}
    (multiple of G=6, with >=1 zero row unless im_l==36 so the reference's
    max-includes-zero clamp is preserved).  Images are rank-sorted by R and
    dealt round-robin to the 8 cores, so all cores share one R "template"
    (SPMD requires identical reduce shapes); rows ~9216 -> ~5300+pad/core... 888.
  * The big einsum runs in fp8 e4m3 with DoubleRow perf mode (K=256 per
    instruction at 0.5 cycles/row): ~4x fewer PE cycles than bf16.
    s is quantized host-side (raw values, |s| << 240); im is normalized on
    device, scaled x16 and cast to fp8; the 1/16 folds into the s scale.
  * No collectives at all: s norms come from the diagonal of per-tile
    fp8 Gram matmuls on the PE (each core loads all s tiles anyway), and
    the final cross-core combine (max/sum over 8 cores' 128x6 payloads)
    happens on the host - that is the gather/unshard step.
  * The max-over-i reduces are split between DVE (direct PSUM reads) and
    GpSimd (via a ScalarE PSUM->SBUF bf16 copy), since DVE alone would be
    the bottleneck at ~1.12 ns/elem.

The device program shape depends only on (im_len, s_len); build_nc is
cached on those layout parameters and recompiled if they change.
"""

import os
import sys

import numpy as np
import ml_dtypes

for _p in ("/opt/trn_rl_repo", "/root/.axon_site/_ro/trn_rl_repo"):
    if os.path.isdir(_p) and _p not in sys.path:
        sys.path.append(_p)

import concourse.bass as bass
import concourse.mybir as mybir
import concourse.tile as tile
from concourse import bacc
from concourse.bass_utils import run_bass_kernel_spmd


def _ensure_axon_hooks():
    """Some agent images ship an ``antenv`` without ``axon_hooks``, but
    bass_utils hard-imports it when trace=True.  Provide the registry and,
    when libaxon_pjrt.so is available, the real NTFF profile hook."""
    import types

    try:
        import antenv.axon_hooks  # noqa: F401
        return
    except ImportError:
        pass
    try:
        import antenv
    except ImportError:
        return
    mod = types.ModuleType("antenv.axon_hooks")
    mod._hook = None
    mod.set_axon_ntff_profile_hook = lambda h: setattr(mod, "_hook", h)
    mod.get_axon_ntff_profile_hook = lambda: mod._hook
    sys.modules["antenv.axon_hooks"] = mod
    antenv.axon_hooks = mod
    so_path = "/opt/axon/libaxon_pjrt.so"
    try:
        import trn_agent_boot.trn_boot as _tb
        if os.path.exists(so_path):
            mod._hook = _tb._ntff_profile_via_ctypes(so_path)
    except Exception:
        pass


_ensure_axon_hooks()

F32 = mybir.dt.float32
F32R = mybir.dt.float32r
BF16 = mybir.dt.bfloat16
F8 = mybir.dt.float8e4
I32 = mybir.dt.int32
AX = mybir.AxisListType
ALU = mybir.AluOpType
ACT = mybir.ActivationFunctionType
DR = mybir.MatmulPerfMode.DoubleRow

NCORES = 8
B, LI, LS, D = 256, 36, 30, 1024
KC = D // 128               # 8 contraction chunks of 128
G = 6                       # im row-padding granularity
MARGIN, EPS, NEG = 0.2, 1e-12, -1.0e9

LAST_RESULT = None  # BassKernelResults of the most recent run (for test harness)

# Dedup redundant PE weight loads: bass lowering splits every matmul into a
# standalone Ldweights + non-self-loading Matmult, but emits one Ldweights
# per matmul even when consecutive matmuls share the same stationary
# operand.  We post-process the BIR json and drop a generated Ldweights
# (no semaphore waits/updates) when the weights signature matches what the
# PE already has loaded.  This is the dominant PE cost for DoubleRow fp8
# (256-column loads at ~112 ns apiece).
LDW_DEDUP = os.environ.get("LDW_DEDUP", "1") == "1"


def _dedup_ldweights_json(js_bytes):
    import json as _json

    j = _json.loads(js_bytes)
    dropped = 0
    for fn in j.get("functions", []):
        for blk in fn.get("blocks", []):
            insts = blk.get("instructions")
            if not insts:
                continue
            out = []
            loaded = None
            for x in insts:
                if x.get("engine") != "PE":
                    out.append(x)
                    continue
                op = x.get("opcode")
                if op == "Ldweights":
                    sig = _json.dumps(
                        [x.get("ins"), x.get("perf_mode"),
                         x.get("tile_size"), x.get("tile_position"),
                         x.get("is_transpose")], sort_keys=True)
                    sync = x.get("sync_info") or {}
                    if (sig == loaded and not sync.get("on_wait")
                            and not sync.get("on_update")):
                        dropped += 1
                        continue
                    loaded = sig
                    out.append(x)
                elif op == "Matmult":
                    if x.get("ldweights") is not False:
                        loaded = None  # self-loading matmul clobbers weights
                    out.append(x)
                else:
                    loaded = None
                    out.append(x)
            blk["instructions"] = out
    return _json.dumps(j).encode(), dropped


# ---------------------------------------------------------------------------
# layout planning (data-dependent, host side)
# ---------------------------------------------------------------------------

class Plan:
    pass


def plan_layout(im_l, s_l):
    p = Plan()
    # ---- s side: per-half compacted row lists ----
    p.NT_h = []
    p.srows = []            # per half: list of (c, j) or None (pad)
    for h in (0, 1):
        rows = [(c, j) for c in range(128 * h, 128 * h + 128)
                for j in range(int(s_l[c]))]
        nt = -(-len(rows) // 128)
        rows = rows + [None] * (nt * 128 - len(rows))
        p.NT_h.append(nt)
        p.srows.append(rows)
    p.NT = p.NT_h[0] + p.NT_h[1]

    # ---- im side: R template shared across cores ----
    R = np.where(im_l >= LI, LI,
                 (G * np.ceil((im_l + 1) / G)).astype(np.int64)).astype(np.int64)
    order = np.argsort(-R, kind="stable")
    p.order = order                       # slot i of core m -> image order[8i+m]
    p.template = [int(R[order[8 * i]]) for i in range(32)]
    # runs of equal R in the (descending) template
    runs = []
    i = 0
    while i < 32:
        j = i
        while j < 32 and p.template[j] == p.template[i]:
            j += 1
        runs.append({"mxoff": i, "n": j - i, "R": p.template[i]})
        i = j
    # pack runs into PSUM banks; the LAST bin reserves 128 cols for the
    # gram matmul (cols 384:512) so the whole per-tile accumulation
    # tensor is nbins banks.  Splitting a run costs one extra segment.
    for nbins in range(2, 9):
        caps = [512] * (nbins - 1) + [384]
        segs = _pack_runs(runs, caps)
        if segs is not None:
            break
    p.nbins = nbins
    p.segs = segs
    for s in segs:
        s["eng"] = "dve"
    p.NR = sum(s["n"] * s["R"] for s in segs)
    p.NRT = -(-p.NR // 128)
    # im slot row offsets: global row index = bin*512... no - rows are
    # packed per-bin contiguously in imt column space: bin bi occupies
    # imt cols [binoff[bi], binoff[bi]+used[bi])
    used = [0] * nbins
    for s in segs:
        used[s["bin"]] = max(used[s["bin"]], s["off"] + s["n"] * s["R"])
    binoff = [0] * nbins
    for bi in range(1, nbins):
        binoff[bi] = binoff[bi - 1] + used[bi - 1]
    p.bin_used, p.bin_off = used, binoff
    p.slot_off = [0] * 32
    for s in segs:
        for t in range(s["n"]):
            p.slot_off[s["mxoff"] + t] = (binoff[s["bin"]] + s["off"]
                                          + t * s["R"])
    return p


def _pack_runs(runs, caps):
    """First-fit-decreasing of template runs into bins with capacities
    ``caps``; runs may split at image boundaries.  Returns segment list
    or None if it does not fit."""
    free = list(caps)
    segs = []
    for r in sorted(runs, key=lambda r: -r["n"] * r["R"]):
        n, R, mxoff = r["n"], r["R"], r["mxoff"]
        # whole-run first fit
        placed = False
        for bi in range(len(caps)):
            if n * R <= free[bi]:
                segs.append({"bin": bi, "off": caps[bi] - free[bi],
                             "n": n, "R": R, "mxoff": mxoff})
                free[bi] -= n * R
                placed = True
                break
        if placed:
            continue
        # split across bins with most free space first
        while n > 0:
            bi = max(range(len(caps)), key=lambda b: free[b])
            k = min(n, free[bi] // R)
            if k == 0:
                return None
            segs.append({"bin": bi, "off": caps[bi] - free[bi],
                         "n": k, "R": R, "mxoff": mxoff})
            free[bi] -= k * R
            n -= k
            mxoff += k
    return segs


def _plan_key(p):
    return (p.NT_h[0], p.NT_h[1], p.nbins, p.NR, tuple(p.bin_used),
            tuple((s["bin"], s["off"], s["n"], s["R"], s["mxoff"], s["eng"])
                  for s in p.segs))


# ---------------------------------------------------------------------------
# device program
# ---------------------------------------------------------------------------

def build_nc(p):
    NT, NT0 = p.NT, p.NT_h[0]
    NBINS, NR, NRT = p.nbins, p.NR, p.NRT

    nc = bacc.Bacc(None, target_bir_lowering=False, debug=False,
                   num_devices=NCORES)

    imr_e = nc.declare_dram_parameter("imr", [NR, D], BF16, isOutput=False)
    st_e = nc.declare_dram_parameter("st", [NT, 128, KC, 128], F8,
                                     isOutput=False)
    gmat_e = nc.declare_dram_parameter("gmat", [128, NT * 128], BF16,
                                       isOutput=False)
    ident_e = nc.declare_dram_parameter("ident", [128, 128], F32,
                                        isOutput=False)
    identbf_e = nc.declare_dram_parameter("identbf", [128, 128], BF16,
                                          isOutput=False)
    pos0_e = nc.declare_dram_parameter("pos0", [128, 32], F32, isOutput=False)
    pos1_e = nc.declare_dram_parameter("pos1", [128, 32], F32, isOutput=False)
    post0_e = nc.declare_dram_parameter("post0", [32, 128], F32, isOutput=False)
    post1_e = nc.declare_dram_parameter("post1", [32, 128], F32, isOutput=False)
    out_e = nc.declare_dram_parameter("out", [128, 6], F32, isOutput=True)

    with tile.TileContext(nc) as tc:
        from contextlib import ExitStack

        with ExitStack() as ctx:
            const = ctx.enter_context(tc.tile_pool(name="const", bufs=1))
            small = ctx.enter_context(tc.tile_pool(name="small", bufs=1))
            stp = ctx.enter_context(tc.tile_pool(name="stp", bufs=3))
            mxp = ctx.enter_context(tc.tile_pool(name="mxp", bufs=NT0 + 4))
            prep = ctx.enter_context(tc.tile_pool(name="prep", bufs=2))
            gsc = ctx.enter_context(tc.tile_pool(name="gsc", bufs=3))
            # PSUM (8 banks): align+gram (NBINS banks) x bufs + S 1 + misc 1
            pal = ctx.enter_context(
                tc.tile_pool(name="pal", bufs=max(2, 6 // NBINS),
                             space="PSUM"))
            pmisc = ctx.enter_context(
                tc.tile_pool(name="pmisc", bufs=1, space="PSUM"))
            psacc = ctx.enter_context(
                tc.tile_pool(name="psacc", bufs=1, space="PSUM"))

            def misc_psum(shape, name):
                return pmisc.tile(shape, F32, tag="misc", bufs=1, name=name)

            # ---- consts ----
            ident = const.tile([128, 128], F32, tag="ident")
            nc.sync.dma_start(out=ident[:, :], in_=ident_e[:, :])
            identbf = const.tile([128, 128], BF16, tag="identbf")
            nc.sync.dma_start(out=identbf[:, :], in_=identbf_e[:, :])

            # ---- im prep: load bf16 rows, l2-normalize, cast fp8, transpose
            imt = const.tile([128, KC * NR], F8, tag="imt")
            imt3 = imt.rearrange("p (k n) -> p k n", k=KC)
            for rt in range(NRT):
                pr = min(128, NR - 128 * rt)
                imr_t = prep.tile([pr, D], BF16, tag="imld", name="imr_t",
                                  bufs=3)
                nc.sync.dma_start(out=imr_t[:, :],
                                  in_=imr_e[128 * rt:128 * rt + pr, :])
                sq = prep.tile([pr, D], BF16, tag="imsq", name="sq")
                ssq = small.tile([pr, 1], F32, tag=f"imssq{rt}", name="ssq")
                nc.scalar.activation(sq[:, :], imr_t[:, :], ACT.Square,
                                     accum_out=ssq[:, :])
                # nrm16 = ||row|| / 16 ; rcp = 16 / ||row||
                nrm16 = small.tile([pr, 1], F32, tag=f"imnrm{rt}", name="nrm")
                nc.scalar.activation(nrm16[:, :], ssq[:, :], ACT.Sqrt,
                                     scale=1.0 / 256.0)
                nc.vector.tensor_scalar_max(nrm16[:, :], nrm16[:, :], EPS)
                rcp = small.tile([pr, 1], F32, tag=f"imrcp{rt}", name="rcp")
                nc.vector.reciprocal(rcp[:, :], nrm16[:, :])
                ims_bf = prep.tile([pr, D], BF16, tag="imsbf", name="ims_bf",
                                   bufs=3)
                nc.vector.tensor_scalar(
                    out=ims_bf[:, :], in0=imr_t[:, :], scalar1=rcp[:, :],
                    scalar2=None, op0=ALU.mult,
                )
                tr_ps = pmisc.tile([128, KC * pr], BF16, tag="misc", bufs=1,
                                   name="tr_ps")
                for k in range(KC):
                    nc.tensor.transpose(
                        tr_ps[:, pr * k:pr * (k + 1)],
                        ims_bf[:, 128 * k:128 * (k + 1)],
                        identbf[0:pr, 0:pr])
                dst = imt3[:, :, 128 * rt:128 * rt + pr]
                src = tr_ps.rearrange("p (k c) -> p k c", k=KC)
                nc.scalar.copy(dst, src)

            # ---- epilogue consts/buffers ----
            gmat = const.tile([128, NT * 128], BF16, tag="gmat")
            nc.sync.dma_start(out=gmat[:, :], in_=gmat_e[:, :])
            pos0 = const.tile([128, 32], F32, tag="pos0")
            nc.sync.dma_start(out=pos0[:, :], in_=pos0_e[:, :])
            pos1 = const.tile([128, 32], F32, tag="pos1")
            nc.sync.dma_start(out=pos1[:, :], in_=pos1_e[:, :])
            post0 = const.tile([32, 128], F32, tag="post0")
            nc.sync.dma_start(out=post0[:, :], in_=post0_e[:, :])
            post1 = const.tile([32, 128], F32, tag="post1")
            nc.sync.dma_start(out=post1[:, :], in_=post1_e[:, :])
            margin128 = const.tile([128, 1], F32, tag="margin128")
            nc.gpsimd.memset(margin128[:, :], MARGIN)

            posm = [pos0, pos1]
            payload = small.tile([128, 6], F32, tag="payload")
            snd = [small.tile([128, 32], F32, tag=f"snd{h}", name=f"snd{h}")
                   for h in range(2)]
            trash = small.tile([128, 128], BF16, tag="trash")
            trash32 = small.tile([128, 32], F32, tag="trash32")
            negm = [small.tile([128, 32], F32, tag=f"negm{h}", name=f"negm{h}")
                    for h in range(2)]
            nc.vector.tensor_scalar_mul(negm[0][:, :], pos0[:, :], NEG)
            nc.vector.tensor_scalar_mul(negm[1][:, :], pos1[:, :], NEG)
            posr = [small.tile([128, 32], F32R, tag=f"posr{h}", name=f"posr{h}")
                    for h in range(2)]
            nc.scalar.copy(posr[0][:, :], pos0[:, :])
            nc.scalar.copy(posr[1][:, :], pos1[:, :])
            postr = [small.tile([32, 128], F32R, tag=f"postr{h}",
                                name=f"postr{h}") for h in range(2)]
            nc.scalar.copy(postr[0][:, :], post0[:, :])
            nc.scalar.copy(postr[1][:, :], post1[:, :])
            rm = small.tile([32, 2], F32, tag="rm")

            # s-norm scratch: sscale_sq[:, t] = ||s_row(p of tile t)||^2
            sscale_sq = small.tile([128, NT], F32, tag="sscale_sq")
            sscale = small.tile([128, NT], F32, tag="sscale")

            # S accumulators: both halves share one PSUM bank
            s_acc = psacc.tile([128, 64], F32, tag="S", name="S")
            s_ps = [s_acc[:, 0:32], s_acc[:, 32:64]]

            mx_tiles = {}

            def emit_tile(t):
                st_t = stp.tile([128, KC * 128], F8, tag="st")
                nc.sync.dma_start(
                    out=st_t.rearrange("p (k c) -> p k c", k=KC),
                    in_=st_e[t, :, :, :],
                )
                st3 = st_t.rearrange("p (k c) -> p k c", k=KC)
                # single PSUM accumulation tensor (NBINS banks): align bins
                # in cols [0, used_bi); gram shares the last bank at 384:512.
                # One tensor keeps the scheduler in k-major emission order so
                # one weight load serves 1+NBINS matmuls after LDW dedup.
                ps_t = pal.tile([128, NBINS * 512], F32, tag="al",
                                name="ps")
                psv = ps_t.rearrange("p (c n) -> p c n", c=NBINS)
                gram = psv[:, NBINS - 1, 384:512]
                ps = [psv[:, bi, 0:p.bin_used[bi]] for bi in range(NBINS)]
                for kp in range(KC // 2):
                    w = st3[:, 2 * kp:2 * kp + 2, :]
                    for bi in range(NBINS):
                        nc.tensor.matmul(
                            ps[bi],
                            lhsT=w,
                            rhs=imt3[:, 2 * kp:2 * kp + 2,
                                     p.bin_off[bi]:p.bin_off[bi]
                                     + p.bin_used[bi]],
                            start=(kp == 0), stop=(kp == KC // 2 - 1),
                            perf_mode=DR, skip_group_check=True,
                        )
                    # gram shares the last bin's bank: PSUM start zeroing is
                    # bank-granular (2KB), so the bin's kp0 start already
                    # marks gram's byte range pending-zero; gram must never
                    # issue its own start or it would re-mark the bank and
                    # drop the bin's kp0 contribution.
                    nc.tensor.matmul(
                        gram, lhsT=w, rhs=w,
                        start=False, stop=(kp == KC // 2 - 1),
                        perf_mode=DR, skip_group_check=True,
                    )
                # max over image rows -> mx [128, 32]
                mx = mxp.tile([128, 32], F32, tag="mx", name="mx")
                for s in p.segs:
                    w = s["n"] * s["R"]
                    src = psv[:, s["bin"], s["off"]:s["off"] + w]
                    if s["eng"] == "dve":
                        nc.vector.tensor_reduce(
                            out=mx[:, s["mxoff"]:s["mxoff"] + s["n"]],
                            in_=src.rearrange("p (n r) -> p n r", r=s["R"]),
                            axis=AX.X, op=ALU.max,
                        )
                    else:
                        cp = gsc.tile([128, w], BF16, tag=f"gsc{s['mxoff']}",
                                      name="cp")
                        nc.scalar.copy(cp[:, :], src)
                        nc.gpsimd.tensor_reduce(
                            out=mx[:, s["mxoff"]:s["mxoff"] + s["n"]],
                            in_=cp.rearrange("p (n r) -> p n r", r=s["R"]),
                            axis=AX.X, op=ALU.max,
                        )
                # gram diag -> sscale_sq[:, t] (after the segment reduces so
                # the in-order DVE never blocks a prior tile's reduces on
                # this tile's matmuls)
                nc.vector.scalar_tensor_tensor(
                    out=trash[:, :], in0=gram, scalar=1.0,
                    in1=ident[:, :], op0=ALU.mult, op1=ALU.mult,
                    accum_out=sscale_sq[:, t:t + 1],
                )
                mx_tiles[t] = mx

            def emit_sscale_batch(t0, n):
                # sscale cols: 1 / (16 * sqrt(q)) = 1 / sqrt(256 q)
                nc.scalar.activation(sscale[:, t0:t0 + n],
                                     sscale_sq[:, t0:t0 + n],
                                     ACT.Sqrt, scale=256.0)
                nc.vector.tensor_scalar_max(sscale[:, t0:t0 + n],
                                            sscale[:, t0:t0 + n], EPS)
                nc.vector.reciprocal(sscale[:, t0:t0 + n],
                                     sscale[:, t0:t0 + n])

            def emit_g_tile(t):
                emit_sscale_batch(t, 1)
                mx_r = small.tile([128, 32], BF16, tag="mx_r",
                                  name="mx_r", bufs=4)
                nc.scalar.mul(mx_r[:, :], mx_tiles[t][:, :],
                              mul=sscale[:, t:t + 1])
                h = 0 if t < NT0 else 1
                t0 = 0 if h == 0 else NT0
                nc.tensor.matmul(
                    s_ps[h],
                    lhsT=gmat[:, 128 * t:128 * (t + 1)],
                    rhs=mx_r[:, :],
                    start=(t == t0), stop=(t == t0 + p.NT_h[h] - 1),
                )

            def emit_stats_h(h):
                # diag extraction: accum_out = sum(S * pos) -> payload col 2+h
                nc.vector.scalar_tensor_tensor(
                    out=trash32[:, :], in0=s_ps[h], scalar=1.0,
                    in1=posm[h][:, :], op0=ALU.mult, op1=ALU.mult,
                    accum_out=payload[:, 2 + h:3 + h],
                )
                nc.vector.tensor_add(snd[h][:, :], s_ps[h], negm[h][:, :])
                nc.vector.tensor_reduce(out=payload[:, h:h + 1],
                                        in_=snd[h][:, :], axis=AX.X,
                                        op=ALU.max)
                stp_ps = misc_psum([32, 128], "stp_ps")
                nc.tensor.transpose(stp_ps[:, :], snd[h][:, :], ident[:, :])
                nc.vector.tensor_reduce(out=rm[:, h:h + 1], in_=stp_ps[:, :],
                                        axis=AX.X, op=ALU.max)

            # ---- main loop (G + its sscale col drained with 2-tile lag) ----
            GLAG = 2
            for t in range(NT):
                emit_tile(t)
                if t - GLAG >= 0:
                    emit_g_tile(t - GLAG)
                if t - GLAG == NT0 - 1:
                    emit_stats_h(0)
            for t in range(max(0, NT - GLAG), NT):
                emit_g_tile(t)
            emit_stats_h(1)

            # ---- row-hinge epilogue ----
            rowmax = small.tile([32, 1], F32, tag="rowmax")
            nc.vector.tensor_max(rowmax[:, :], rm[:, 0:1], rm[:, 1:2])
            # own-diag per image (row order): for each half h, pos_h^T @ d_h
            dca = small.tile([128, 2], F32R, tag="dca")
            dcb = small.tile([128, 2], F32R, tag="dcb")
            nc.scalar.copy(dca[:, 0:1], payload[:, 2:3])
            nc.scalar.mul(dca[:, 1:2], payload[:, 2:3], mul=0.0)
            nc.scalar.copy(dcb[:, 0:1], payload[:, 3:4])
            nc.scalar.mul(dcb[:, 1:2], payload[:, 3:4], mul=0.0)
            dfree_ps = misc_psum([32, 2], "dfree_ps")
            nc.tensor.matmul(dfree_ps[:, :], lhsT=posr[0][:, :],
                             rhs=dca[:, :], start=True, stop=False)
            nc.tensor.matmul(dfree_ps[:, :], lhsT=posr[1][:, :],
                             rhs=dcb[:, :], start=False, stop=True)
            dfree_sb = small.tile([32, 1], F32, tag="dfree_sb")
            nc.scalar.copy(dfree_sb[:, :], dfree_ps[:, 0:1])
            rh_pre = small.tile([32, 2], F32, tag="rh_pre")
            nc.gpsimd.memset(rh_pre[:, :], 0.0)
            nc.vector.tensor_sub(rh_pre[:, 0:1], rowmax[:, :], dfree_sb[:, :])
            rowhinge = small.tile([32, 2], F32R, tag="rowhinge")
            nc.scalar.activation(rowhinge[:, :], rh_pre[:, :], ACT.Relu,
                                 bias=margin128[0:32, :])
            for h in range(2):
                rh_ps = misc_psum([128, 2], "rh_ps")
                nc.tensor.matmul(rh_ps[:, :], lhsT=postr[h][:, :],
                                 rhs=rowhinge[:, :], start=True, stop=True)
                nc.scalar.copy(payload[:, 4 + h:5 + h], rh_ps[:, 0:1])

            nc.sync.dma_start(out=out_e[:, :], in_=payload[:, :])

    nc.finalize()
    return nc


# ---------------------------------------------------------------------------
# host side
# ---------------------------------------------------------------------------

def build_in_maps(p, im_set, s_seq):
    im_set = np.asarray(im_set, dtype=np.float32)
    s_seq = np.asarray(s_seq, dtype=np.float32)
    NT, NT0, NR = p.NT, p.NT_h[0], p.NR

    # s tiles (shared): fp8 of raw word rows in compacted order
    s8 = np.zeros((NT * 128, D), dtype=np.float32)
    gmat = np.zeros((128, NT * 128), dtype=np.float32)
    for h in (0, 1):
        base = 0 if h == 0 else NT0 * 128
        for i, cj in enumerate(p.srows[h]):
            if cj is None:
                continue
            c, j = cj
            s8[base + i] = s_seq[c, 1 + j]
            t, pp = divmod(base + i, 128)
            gmat[pp, 128 * t + (c - 128 * h)] = 1.0
    s8 = np.clip(s8, -240.0, 240.0).astype(ml_dtypes.float8_e4m3)
    gmat = gmat.astype(ml_dtypes.bfloat16)
    st = np.ascontiguousarray(
        s8.reshape(NT, 128, KC, 128).transpose(0, 3, 2, 1))

    ident = np.eye(128, dtype=np.float32)
    identbf = ident.astype(ml_dtypes.bfloat16)

    in_maps = []
    for m in range(NCORES):
        imr = np.zeros((NR, D), dtype=np.float32)
        pos0 = np.zeros((128, 32), np.float32)
        pos1 = np.zeros((128, 32), np.float32)
        for i in range(32):
            b = int(p.order[8 * i + m])
            off = p.slot_off[i]
            nvalid = int(p.im_l[b])
            imr[off:off + nvalid] = im_set[b, 1:1 + nvalid]
            if b < 128:
                pos0[b % 128, i] = 1.0
            else:
                pos1[b % 128, i] = 1.0
        in_maps.append({
            "imr": imr.astype(ml_dtypes.bfloat16),
            "st": st,
            "gmat": gmat,
            "ident": ident,
            "identbf": identbf,
            "pos0": pos0,
            "pos1": pos1,
            "post0": np.ascontiguousarray(pos0.T),
            "post1": np.ascontiguousarray(pos1.T),
        })
    return in_maps


def host_combine(outs):
    """Combine the 8 cores' [128, 6] payloads into the scalar loss."""
    agg = np.stack([np.asarray(o, dtype=np.float32) for o in outs])  # [8,128,6]
    colmax = agg[:, :, 0:2].max(axis=0)          # [128, 2]
    diag = agg[:, :, 2:4].sum(axis=0)            # [128, 2]
    colhinge = np.maximum(MARGIN + colmax - diag, 0.0).sum()
    rowhinge = agg[:, :, 4:6].sum()
    return np.float32(colhinge + rowhinge)


_NC_CACHE = {}


def kernel(im_set, s_seq, im_len, s_len):
    global LAST_RESULT
    im_len = np.asarray(im_len, dtype=np.int32)
    s_len = np.asarray(s_len, dtype=np.int32)
    im_l = im_len - 1
    s_l = s_len - 3

    p = plan_layout(im_l, s_l)
    p.im_l = im_l
    key = _plan_key(p)
    if key not in _NC_CACHE:
        nc = build_nc(p)
        if LDW_DEDUP:
            _orig = nc.to_json_bytes

            def _to_json_bytes_dedup(_orig=_orig):
                js, _ = _dedup_ldweights_json(_orig())
                return js

            nc.to_json_bytes = _to_json_bytes_dedup
        _NC_CACHE[key] = nc
    nc = _NC_CACHE[key]

    in_maps = build_in_maps(p, im_set, s_seq)
    res = run_bass_kernel_spmd(nc, in_maps, core_ids=list(range(NCORES)))
    LAST_RESULT = res
    return host_combine([r["out"] for r in res.results])
